# revision 40
# baseline (speedup 1.0000x reference)
"""CGConv-style GNN encoder (2 x (CGConv + BatchNorm) + global mean pool) on
8 TRN2 NeuronCores.

Sharding: nodes are padded and split into 8 contiguous per-core shards; each
edge is owned by the core that owns its dst node, so the scatter-add is core
local.  Small weights are replicated; only BatchNorm statistics are
all-reduced (layer 1; layer 2's BN is folded into the pooled output on the
host, which is exact because pooling is linear).  The updated node features
are all-gathered in bf16 between the layers.

Per 128-edge tile on device:
  - x[src]/x[dst] rows are fetched with dma_gather(transpose=True) from bf16
    tables in internal DRAM, landing channel-major [128c, 128e].  Tables are
    kept below 8MB inside 16MB-aligned windows (the gather ucode corrupts
    addresses whose byte offset has bits 23 and 15 both set).
  - Three N=256 matmuls (dst rows, src rows, edge_attr+bias row) accumulate
    z @ [-Wf | Ws] into a PSUM slab [128e, SLAB*256].  The f-half weights are
    negated on the host so one ACT Exp over the whole slab yields
    [exp(-f) | exp(s)]; softplus(s) = Ln(exp(s) + 1) via the ACT bias input,
    sigmoid(f) = recip_approx_fast(1 + exp(-f)) on the DVE.  Only Exp/Ln are
    used, which both live in the natural_log_exp ACT table - zero mid-loop
    ACT_TABLE_LOADs.
  - The scatter-add is a matmul with a one-hot matrix U[e, n] built by a
    DVE tensor_scalar is_equal; the dst-select one-hot UT[n, e] is built
    once per gather group (batched) the same way.
"""

import numpy as np
import ml_dtypes

import concourse.bacc as bacc
import concourse.mybir as mybir
import concourse.tile as tile
from concourse.bass_utils import run_bass_kernel_spmd
from contextlib import ExitStack

F32 = mybir.dt.float32
BF16 = mybir.dt.bfloat16
I16 = mybir.dt.int16
AF = mybir.ActivationFunctionType
ALU = mybir.AluOpType
BF16NP = ml_dtypes.bfloat16

NCORES = 8
D = 128


def _setup_act_tables():
    """Reorder act_info.json so Exp/Ln/Copy all resolve to one table set
    (natural_log_exp_and_others), eliminating per-op ACT_TABLE_LOADs."""
    import json, shutil
    import concourse.hw_specs as hw_specs
    import concourse.bacc as _bacc
    from neuronxcc.driver.Job import Job
    from neuronxcc.driver.jobs.support.FindActInfo import findActInfoFile

    if os.environ.get("BASS_ACT_ROOT_JSON_PATH"):
        return
    src = findActInfoFile(Job.getPackageDir(), "gen3")
    dstdir = "/tmp/act_custom"
    os.makedirs(dstdir, exist_ok=True)
    d = json.load(open(src))
    order = sorted(range(len(d["act_func_sets"])),
                   key=lambda i: d["act_func_sets"][i]["name"] != "natural_log_exp_and_others")
    d["act_func_sets"] = [d["act_func_sets"][i] for i in order]
    with open(os.path.join(dstdir, "act_info.json"), "w") as f:
        json.dump(d, f)
    srcdir = os.path.dirname(src)
    for fn in os.listdir(srcdir):
        if fn != "act_info.json":
            tgt = os.path.join(dstdir, fn)
            if not os.path.exists(tgt):
                os.symlink(os.path.join(srcdir, fn), tgt)
    os.environ["BASS_ACT_ROOT_JSON_PATH"] = os.path.join(dstdir, "act_info.json")

    import concourse.mybir as _mybir
    def _tables(arch, _d=d):
        return {
            ent["name"]: {
                _mybir.ActivationFunctionType.from_pwp(v) for v in ent["act"].keys()
            }
            for ent in _d["act_func_sets"]
        }
    hw_specs.get_activation_tables = _tables
    _bacc.get_activation_tables = _tables


import os
_setup_act_tables()
DE = 64
EPS = 1e-5
SLAB = 4  # tiles per activation slab


def _ceil(a, b):
    return -(-a // b)


# ---------------------------------------------------------------------------
# host-side data prep
# ---------------------------------------------------------------------------

def _prep(x, edge_index, edge_attr, batch, G):
    N = x.shape[0]
    NBC = _ceil(_ceil(N, 128), NCORES)
    NPC = NBC * 128
    NP = NPC * NCORES
    HALF = NP // 2

    src = np.asarray(edge_index[0], np.int64)
    dst = np.asarray(edge_index[1], np.int64)
    ea = np.asarray(edge_attr, np.float32)
    batch = np.asarray(batch, np.int64)

    HA = 3200          # rows per core in the A half (25 blocks)
    HBS = NPC - HA + 4  # B-half stride per core (24 blocks + 4 stats rows)
    core_of = dst // NPC
    dst_loc = dst - core_of * NPC
    blk = dst_loc >> 7
    src_core = src // NPC
    src_loc = src - src_core * NPC
    half = (src_loc >= HA).astype(np.int64)

    keys = core_of * (NBC * 2) + blk * 2 + half
    order = np.lexsort((src, keys))
    src_s, dstl_s = src[order], dst_loc[order]
    ea_s = ea[order]
    keys_s = keys[order]

    counts = np.zeros((NCORES, NBC, 2), np.int64)
    np.add.at(counts, (core_of, blk, half), 1)
    m = _ceil(counts.max(axis=0), 128)  # [NBC, 2] tiles per group
    T = int(m.sum())
    goff = np.zeros((NBC, 2), np.int64)  # group start, in edges
    acc = 0
    for b in range(NBC):
        for h in (0, 1):
            goff[b, h] = acc
            acc += m[b, h] * 128

    idx_src = np.zeros((NCORES, T * 128), np.int64)
    dstcol = np.full((NCORES, T * 128), 1024.0, np.float32)
    ea_flat = np.zeros((NCORES, T * 128 * 66), BF16NP)

    bounds = np.searchsorted(keys_s, np.arange(NCORES * NBC * 2 + 1))
    for c in range(NCORES):
        for b in range(NBC):
            for h in (0, 1):
                k = c * (NBC * 2) + b * 2 + h
                lo, hi = bounds[k], bounds[k + 1]
                n = hi - lo
                o = int(goff[b, h])
                L = int(m[b, h]) * 128
                sc = src_s[lo:hi] // NPC
                sl = src_s[lo:hi] - sc * NPC
                idx_src[c, o:o + n] = np.where(
                    sl < HA, sc * HA + sl, sc * HBS + (sl - HA))
                dstcol[c, o:o + n] = (dstl_s[lo:hi] - b * 128).astype(np.float32)
                # group ea block [66, L] at flat offset 66*o:
                # row 64 = 1.0 (bias), row 65 = dst-in-block (1024 for pads)
                blkea = np.zeros((66, L), np.float32)
                blkea[:64, :n] = ea_s[lo:hi].T
                blkea[64, :] = 1.0
                blkea[65, :] = 1024.0
                blkea[65, :n] = (dstl_s[lo:hi] - b * 128).astype(np.float32)
                ea_flat[c, 66 * o: 66 * (o + L)] = blkea.astype(BF16NP).ravel()

    def wrap16(v):  # [T*128] -> [128, T*8]
        return np.tile(v.reshape(-1, 16).T.astype(np.int16), (8, 1))

    prep = dict(N=N, NBC=NBC, NPC=NPC, NP=NP, HALF=HALF, m=m, T=T, goff=goff,
                HA=HA, HBS=HBS)
    prep["idx_src_w"] = np.stack([wrap16(idx_src[c]) for c in range(NCORES)])
    prep["ea_flat"] = ea_flat
    # scatter one-hots, e-major tiles packed column-wise:
    # u_flat[c, e, t*128 + p] = (dst_in_block(edge t*128+e) == p)
    dst3 = dstcol.reshape(NCORES, T, 128)
    u4 = (dst3[:, :, :, None] == np.arange(128, dtype=np.float32)).astype(BF16NP)
    prep["u_flat"] = np.ascontiguousarray(
        u4.transpose(0, 2, 1, 3)).reshape(NCORES, 128, T * 128)
    # p-major dst-select one-hots: ut2[p, t*128+e] = (dst(t*128+e) == p)
    prep["ut_flat"] = np.ascontiguousarray(
        u4.transpose(0, 3, 1, 2)).reshape(NCORES, 128, T * 128)
    prep["iotaB"] = np.broadcast_to(
        np.arange(128, dtype=np.float32)[:, None].astype(BF16NP),
        (128, int(m.max()) * 128)).copy()

    xpad = np.zeros((NP, D), np.float32)
    xpad[:N] = np.asarray(x, np.float32)
    prep["x_shard"] = xpad.reshape(NCORES, NPC, D).copy()
    prep["x_shard_bf"] = prep["x_shard"].astype(BF16NP)
    # full gather tables, replicated to every core via the input map
    prep["xAt"] = np.ascontiguousarray(
        prep["x_shard_bf"][:, :HA]).reshape(NCORES * HA, D)
    xBt = np.zeros((NCORES, HBS, D), BF16NP)
    xBt[:, :NPC - HA] = prep["x_shard_bf"][:, HA:]
    prep["xBt"] = xBt.reshape(NCORES * HBS, D)

    ids = np.arange(NP).reshape(NCORES, NBC, 128)
    prep["mask"] = np.ascontiguousarray(
        (ids < N).astype(np.float32).transpose(0, 2, 1))

    gnode = np.full(NP, -1, np.int64)
    gnode[:N] = batch
    glo = np.zeros(NCORES, np.int64)
    GW = 1
    for c in range(NCORES):
        gs = gnode[c * NPC:(c + 1) * NPC]
        gs = gs[gs >= 0]
        if len(gs):
            glo[c] = int(gs.min())
            GW = max(GW, int(gs.max() - gs.min() + 1))
    Bmat = np.zeros((NCORES, NBC * 128, 1 + GW), np.float32)
    Bmat[:, :, 0] = 1.0  # stats row: sums all nodes (pads have y=0)
    for c in range(NCORES):
        gs = gnode[c * NPC:(c + 1) * NPC]
        rows = np.nonzero(gs >= 0)[0]
        Bmat[c, rows, 1 + gs[rows] - glo[c]] = 1.0
    prep["Bmat"], prep["GW"], prep["glo"] = Bmat, GW, glo
    prep["cnts"] = np.bincount(batch, minlength=G).astype(np.float32)
    return prep


def _wcat(Wf, bf, Ws, bs):
    # f-half negated: the slab then holds [-f | s] so one Exp gives
    # [exp(-f) | exp(s)].
    Wf = -np.asarray(Wf, np.float32)
    Ws = np.asarray(Ws, np.float32)
    wd = np.concatenate([Wf[0:D], Ws[0:D]], axis=1)
    ws = np.concatenate([Wf[D:2 * D], Ws[D:2 * D]], axis=1)
    we = np.zeros((65, 2 * D), np.float32)
    we[:64] = np.concatenate([Wf[2 * D:], Ws[2 * D:]], axis=1)
    we[64, :D] = -np.asarray(bf, np.float32)
    we[64, D:] = np.asarray(bs, np.float32)
    return (wd.astype(BF16NP), ws.astype(BF16NP), we.astype(BF16NP))


# ---------------------------------------------------------------------------
# device program
# ---------------------------------------------------------------------------

def _build(prep, debug=False):
    N = prep["N"]
    NBC, NPC, NP, HALF = prep["NBC"], prep["NPC"], prep["NP"], prep["HALF"]
    m, T, goff, GW = prep["m"], prep["T"], prep["goff"], prep["GW"]
    Lmax = int(m.max()) * 128

    HA, HBS = prep["HA"], prep["HBS"]
    NA, NB = NCORES * HA, NCORES * HBS

    nc = bacc.Bacc("TRN2", target_bir_lowering=False, debug=False,
                   num_devices=NCORES, num_swdge_queues=4)

    # ---- I/O
    x_in = nc.dram_tensor("x_shard", [NPC, D], F32, kind="ExternalInput")
    xbf_in = nc.dram_tensor("x_shard_bf", [NPC, D], BF16, kind="ExternalInput")
    xat_in = nc.dram_tensor("xAt", [NA, D], BF16, kind="ExternalInput")
    xbt_in = nc.dram_tensor("xBt", [NB, D], BF16, kind="ExternalInput")
    isrc = nc.dram_tensor("idx_src", [128, T * 8], I16, kind="ExternalInput")
    eain = nc.dram_tensor("ea_flat", [T * 128 * 66], BF16, kind="ExternalInput")
    uin = nc.dram_tensor("u_flat", [128, T * 128], BF16, kind="ExternalInput")
    utin = nc.dram_tensor("ut_flat", [128, T * 128], BF16, kind="ExternalInput")
    mask_in = nc.dram_tensor("mask", [128, NBC], F32, kind="ExternalInput")
    bmat_in = nc.dram_tensor("Bmat", [NBC * 128, 1 + GW], F32, kind="ExternalInput")
    w_ins = []
    for l in range(2):
        w_ins.append((
            nc.dram_tensor(f"wd{l}", [128, 256], BF16, kind="ExternalInput"),
            nc.dram_tensor(f"ws{l}", [128, 256], BF16, kind="ExternalInput"),
            nc.dram_tensor(f"we{l}", [65, 256], BF16, kind="ExternalInput"),
        ))
    bng_in = nc.dram_tensor("bn_g", [1, D], F32, kind="ExternalInput")
    bnb_in = nc.dram_tensor("bn_b", [1, D], F32, kind="ExternalInput")

    pool_out = nc.dram_tensor("pool_out", [GW, D], F32, kind="ExternalOutput")
    stats2_out = nc.dram_tensor("stats2", [1, 256], F32, kind="ExternalOutput")
    if debug:
        dbg_xoT = nc.dram_tensor("dbg_xoT", [128, 256], F32, kind="ExternalOutput")
        dbg_P = nc.dram_tensor("dbg_P", [128, 256], F32, kind="ExternalOutput")
        dbg_UT = nc.dram_tensor("dbg_UT", [128, 256], F32, kind="ExternalOutput")
        dbg_gs = nc.dram_tensor("dbg_gs", [128, 256], F32, kind="ExternalOutput")
        dbg_ee = nc.dram_tensor("dbg_ee", [128, 512], F32, kind="ExternalOutput")
        dbg_msg = nc.dram_tensor("dbg_msg", [128, 512], F32, kind="ExternalOutput")
        dbg_agg = nc.dram_tensor("dbg_agg", [128, 128], F32, kind="ExternalOutput")
        dbg_y = nc.dram_tensor("dbg_y", [128, 128], F32, kind="ExternalOutput")
        dbg_arv = nc.dram_tensor("dbg_arv", [1, 256], F32, kind="ExternalOutput")
        dbg_st8 = nc.dram_tensor("dbg_st8", [1, 4096], BF16, kind="ExternalOutput")
        dbg_stsb = nc.dram_tensor("dbg_stsb", [1, 256], F32, kind="ExternalOutput")

    # ---- internal DRAM: gather tables below 8MB of 16MB-aligned windows
    WIN = 16 * 1024 * 1024
    SAFE = 8 * 1024 * 1024

    def win_tables(specs, shared=False):
        out = []
        space = "Shared" if shared else "Local"
        pos = nc.shared_dram_base if shared else nc.local_dram_base
        pad = (-pos) % WIN
        if pad:
            nc.dram_tensor(f"_pad{specs[0][0]}", [pad], mybir.dt.uint8,
                           addr_space=space)
        used = 0
        for name, rows in specs:
            nbytes = rows * D * 2
            assert used + nbytes <= SAFE, (name, rows)
            out.append(nc.dram_tensor(name, [rows, D], BF16, addr_space=space))
            used += nbytes
            used += (-used) % 4096
        return out

    # AllGather outputs double as the gather tables (saves the 2x HBM
    # traffic of staging copies); each in its own 16MB window
    (xA,) = win_tables([("xA", NA)], shared=True)
    (xB,) = win_tables([("xB", NB)], shared=True)
    (yA,) = win_tables([("yA", NA)], shared=True)
    (yB,) = win_tables([("yB", NB)], shared=True)

    y_ownA = nc.dram_tensor("y_ownA", [HA, D], BF16)
    y_ownB = nc.dram_tensor("y_ownB", [HBS, D], BF16)
    y1_res = nc.dram_tensor("y1_res", [NPC, D], F32)
    rowscr = nc.dram_tensor("rowscr", [2, 128], F32)
    stats_scr = nc.dram_tensor("stats_scr", [1, 512], BF16)
    rg = [list(range(NCORES))]

    with tile.TileContext(nc) as tc, ExitStack() as ctx:
        const = ctx.enter_context(tc.tile_pool(name="const", bufs=1))
        gat = ctx.enter_context(tc.tile_pool(name="gat", bufs=4))
        work = ctx.enter_context(tc.tile_pool(name="work", bufs=3))
        blkp = ctx.enter_context(tc.tile_pool(name="blkp", bufs=3))
        ps_slab = ctx.enter_context(tc.tile_pool(name="ps_slab", bufs=2, space="PSUM"))
        ps_agg = ctx.enter_context(tc.tile_pool(name="ps_agg", bufs=1, space="PSUM"))
        ps_misc = ctx.enter_context(tc.tile_pool(name="ps_misc", bufs=1, space="PSUM"))
        ps_tr = ctx.enter_context(tc.tile_pool(name="ps_tr", bufs=1, space="PSUM"))

        # ---- prologue: copy host-provided full x tables into the
        # gather-safe windows (no collectives needed for x)
        nc.sync.dma_start(out=xA[:, :], in_=xat_in[:, :])
        nc.sync.dma_start(out=xB[:, :], in_=xbt_in[:, :])

        # ---- constants and resident streams
        W = []
        for l in range(2):
            wd = const.tile([128, 256], BF16, tag=f"wd{l}")
            ws = const.tile([128, 256], BF16, tag=f"ws{l}")
            we = const.tile([65, 256], BF16, tag=f"we{l}")
            nc.sync.dma_start(out=wd[:], in_=w_ins[l][0][:])
            nc.sync.dma_start(out=ws[:], in_=w_ins[l][1][:])
            nc.sync.dma_start(out=we[:], in_=w_ins[l][2][:])
            W.append((wd, ws, we))
        ones_r = const.tile([1, 128], F32, tag="ones_r")
        nc.gpsimd.memset(ones_r[:], 1.0)
        eps_r = const.tile([1, 1], F32, tag="eps_r")
        nc.gpsimd.memset(eps_r[:], EPS)
        neghalf_r = const.tile([1, 1], F32, tag="neghalf_r")
        nc.gpsimd.memset(neghalf_r[:], -0.5)
        from concourse.masks import make_identity
        ident_t = const.tile([128, 128], BF16, tag="ident")
        make_identity(nc, ident_t[:])
        bng_t = const.tile([1, D], F32, tag="bng")
        nc.sync.dma_start(out=bng_t[:], in_=bng_in[:])
        bnb_t = const.tile([1, D], F32, tag="bnb")
        nc.sync.dma_start(out=bnb_t[:], in_=bnb_in[:])
        mask_t = const.tile([128, NBC], F32, tag="mask")
        nc.sync.dma_start(out=mask_t[:], in_=mask_in[:])
        is_t = const.tile([128, T * 8], I16, tag="is")
        nc.sync.dma_start(out=is_t[:], in_=isrc[:])
        S1_t = const.tile([128, 128], F32, tag="S1")
        B1_t = const.tile([128, 128], F32, tag="B1")


        # ---------------- layer body ----------------
        self_qn = [0]

        def layer(l, tblA, tblB, tbl_ownA, tbl_ownB=None):
            wd, ws, we = W[l]
            # own-shard transposed features, resident for the layer
            # (per-block plain DMA + PE transpose; avoids Q7 gathers and the
            # HWDGE xbar which conflicts with in-flight xbar gathers)
            xoT = const.tile([128, NPC], BF16, tag="xoT")
            stats_ps = statpool[0:1, 0:256]
            for b in range(NBC):
                xb_ = blkp.tile([128, 128], BF16, tag="xb_")
                if tbl_ownB is None:
                    src_ap = tbl_ownA[b * 128:(b + 1) * 128, :]
                elif b < 25:
                    src_ap = tbl_ownA[b * 128:(b + 1) * 128, :]
                else:
                    src_ap = tbl_ownB[(b - 25) * 128:(b - 24) * 128, :]
                nc.sync.dma_start(out=xb_[:], in_=src_ap)
                ptr = ps_tr.tile([128, 128], BF16, tag="ptr")
                nc.tensor.transpose(out=ptr[:], in_=xb_[:], identity=ident_t[:])
                nc.vector.tensor_copy(xoT[:, b * 128:(b + 1) * 128], ptr[:])
            for b in range(NBC):
                agg = ps_agg.tile([128, 128], F32, tag="agg")
                # per-block dst-projection table P = x_block @ [-Wf|Ws]_dst
                pps = ps_misc.tile([128, 256], F32, tag="pps")
                nc.tensor.matmul(pps[:], lhsT=xoT[:, b * 128:(b + 1) * 128],
                                 rhs=wd[:], start=True, stop=True)
                P_sb = blkp.tile([128, 256], BF16, tag="P_sb")
                nc.vector.tensor_copy(P_sb[:], pps[:])
                if debug and l == 0 and b == 0:
                    dt_ = blkp.tile([128, 256], F32, tag="dt_")
                    nc.vector.tensor_copy(dt_[:], P_sb[:])
                    nc.sync.dma_start(out=dbg_P[:], in_=dt_[:])
                    dt2_ = blkp.tile([128, 256], F32, tag="dt2_")
                    nc.vector.tensor_copy(dt2_[:], xoT[:, 0:256])
                    nc.sync.dma_start(out=dbg_xoT[:], in_=dt2_[:])
                tiles = []  # (tile_global, (gs, UTg, eat), col_in_gather)
                for h in (0, 1):
                    mb = int(m[b, h])
                    if mb == 0:
                        continue
                    L = mb * 128
                    o = int(goff[b, h])
                    gtbl = tblA if h == 0 else tblB
                    GCH = Lmax  # one multi-packet gather per (block, half) group
                    gs = gat.tile([128, Lmax], BF16, tag="gs")
                    for q0 in range(0, L, GCH):
                        q1 = min(q0 + GCH, L)
                        if self_qn[0] % 2 == 0:
                            # xbar transpose-gather on queue 0
                            nc.gpsimd.dma_gather(
                                out_ap=gs[:, q0:q1].rearrange("p (o k) -> p o k", o=1),
                                in_ap=gtbl[:, :],
                                idxs_ap=is_t[:, (o + q0) // 16:(o + q1) // 16],
                                num_idxs=q1 - q0, num_idxs_reg=q1 - q0,
                                elem_size=D, transpose=True, queue_num=0,
                                single_packet=False)
                        else:
                            # plain row gather on queue 1 (no xbar) + PE transpose
                            nsub = (q1 - q0 + 127) // 128
                            gp = gat.tile([128, (Lmax + 127) // 128, 128], BF16, tag="gp")
                            nc.gpsimd.dma_gather(
                                out_ap=gp[:, :nsub, :],
                                in_ap=gtbl[:, :],
                                idxs_ap=is_t[:, (o + q0) // 16:(o + q1) // 16],
                                num_idxs=q1 - q0, num_idxs_reg=q1 - q0,
                                elem_size=D, transpose=False, queue_num=1,
                                single_packet=False)
                            for jt in range(nsub):
                                ptr = ps_tr.tile([128, 128], BF16, tag="ptr")
                                nc.tensor.transpose(
                                    out=ptr[:], in_=gp[:, jt, :], identity=ident_t[:])
                                nc.vector.tensor_copy(
                                    gs[:, q0 + jt * 128:q0 + (jt + 1) * 128], ptr[:])
                        self_qn[0] += 1
                    eat = gat.tile([66, Lmax], BF16, tag="eat")
                    nc.sync.dma_start(
                        out=eat[:, :L],
                        in_=eain[66 * o:66 * (o + L)].rearrange("(p e) -> p e", p=66))
                    # host-built one-hots: UTg[p, e] dst-select (lhsT of the
                    # projection matmul), ug[e, tile*128+p] scatter (lhsT of
                    # the aggregation matmul)
                    UTg = gat.tile([128, Lmax], BF16, tag="UTg")
                    nc.sync.dma_start(out=UTg[:, :L], in_=utin[:, o:o + L])
                    ug = gat.tile([128, Lmax], BF16, tag="ug")
                    nc.sync.dma_start(out=ug[:, :L], in_=uin[:, o:o + L])
                    if debug and l == 0 and b == 0 and h == 0:
                        du_ = blkp.tile([128, 256], F32, tag="du_")
                        nc.vector.tensor_copy(du_[:], UTg[:, 0:256])
                        nc.sync.dma_start(out=dbg_UT[:], in_=du_[:])
                        dg_ = blkp.tile([128, 256], F32, tag="dg_")
                        nc.vector.tensor_copy(dg_[:], gs[:, 0:256])
                        nc.sync.dma_start(out=dbg_gs[:], in_=dg_[:])
                    for j in range(mb):
                        tiles.append((o // 128 + j, (gs, UTg, eat, ug), j))
                nt = len(tiles)
                for s0 in range(0, nt, SLAB):
                    ns = min(SLAB, nt - s0)
                    nf = ns * 128
                    slab = ps_slab.tile([128, SLAB * 256], F32, tag="slab")
                    for jj in range(ns):
                        tg, (gs, UTg, eat, ug), j = tiles[s0 + jj]
                        osl = slice(jj * 256, jj * 256 + 256)
                        esl = slice(j * 128, (j + 1) * 128)
                        nc.tensor.matmul(slab[:, osl], lhsT=UTg[:, esl], rhs=P_sb[:],
                                         start=True, stop=False)
                        nc.tensor.matmul(slab[:, osl], lhsT=gs[:, esl], rhs=ws[:],
                                         start=False, stop=False)
                        nc.tensor.matmul(slab[:, osl], lhsT=eat[:65, esl], rhs=we[:],
                                         start=False, stop=True)
                    # ee = [exp(-f) | exp(s)] over the whole slab
                    ee = work.tile([128, SLAB * 256], BF16, tag="ee")
                    nc.scalar.activation(ee[:, :ns * 256], slab[:, :ns * 256], AF.Exp)
                    # l2 = ln(ee + 1) = [softplus(-f) | softplus(s)]
                    l2 = work.tile([128, SLAB * 256], BF16, tag="l2")
                    nc.scalar.activation(l2[:, :ns * 256], ee[:, :ns * 256],
                                         AF.Ln, bias=1.0)
                    l23 = l2[:, :ns * 256].rearrange("p (t c) -> p t c", c=256)
                    # sigmoid(f) = exp(-softplus(-f))
                    sg = work.tile([128, SLAB * 128], BF16, tag="sg")
                    sg3 = sg[:, :nf].rearrange("p (t c) -> p t c", c=128)
                    nc.scalar.activation(sg3, l23[:, :, 0:128], AF.Exp, scale=-1.0)
                    msg = work.tile([128, SLAB * 128], BF16, tag="msg")
                    nc.vector.tensor_tensor(out=msg[:, :nf], in0=sg[:, :nf],
                                            in1=l23[:, :, 128:256], op=ALU.mult)
                    if debug and l == 0 and b == 0 and s0 == 0:
                        de_ = blkp.tile([128, 512], F32, tag="de_")
                        nc.vector.tensor_copy(de_[:], ee[:, 0:512])
                        nc.sync.dma_start(out=dbg_ee[:], in_=de_[:])
                        dm_ = blkp.tile([128, 512], F32, tag="dm_")
                        nc.vector.tensor_copy(dm_[:], msg[:, 0:512])
                        nc.sync.dma_start(out=dbg_msg[:], in_=dm_[:])
                    for jj in range(ns):
                        tg, (gs2, UTg2, eat2, ug2), j2 = tiles[s0 + jj]
                        nc.tensor.matmul(
                            agg[:], lhsT=ug2[:, j2 * 128:(j2 + 1) * 128],
                            rhs=msg[:, jj * 128:(jj + 1) * 128],
                            start=(s0 + jj == 0), stop=(s0 + jj == nt - 1))
                # ---- block epilogue
                if debug and l == 0 and b == 0:
                    da_ = blkp.tile([128, 128], F32, tag="da_")
                    nc.vector.tensor_copy(da_[:], agg[:])
                    nc.sync.dma_start(out=dbg_agg[:], in_=da_[:])
                xres = blkp.tile([128, D], F32, tag="xres")
                if l == 0:
                    nc.sync.dma_start(out=xres[:], in_=x_in[b * 128:(b + 1) * 128, :])
                    xeff = xres
                else:
                    nc.sync.dma_start(out=xres[:], in_=y1_res[b * 128:(b + 1) * 128, :])
                    xe1 = blkp.tile([128, D], F32, tag="xe1")
                    nc.vector.tensor_tensor(out=xe1[:], in0=xres[:], in1=S1_t[:],
                                            op=ALU.mult)
                    xeff = blkp.tile([128, D], F32, tag="xe2")
                    nc.vector.tensor_tensor(out=xeff[:], in0=xe1[:], in1=B1_t[:],
                                            op=ALU.add)
                ysum = blkp.tile([128, D], F32, tag="ysum")
                nc.vector.tensor_tensor(out=ysum[:], in0=xeff[:], in1=agg[:],
                                        op=ALU.add)
                yy = blkp.tile([128, 2 * D], F32, tag="yy")
                y = yy[:, 0:D]
                nc.vector.tensor_scalar_mul(y, ysum[:], mask_t[:, b:b + 1])
                nc.vector.tensor_tensor(out=yy[:, D:2 * D], in0=y, in1=y,
                                        op=ALU.mult)
                # fused stats+pool: row 0 = [sum y | sum y^2], rows 1..GW =
                # [pool | pool-of-y^2 (unused)].  One accumulation group so
                # nothing else touches this bank's has_written bits.
                bm = blkp.tile([128, 1 + GW], F32, tag="bm")
                nc.sync.dma_start(out=bm[:], in_=bmat_in[b * 128:(b + 1) * 128, :])
                nc.tensor.matmul(statpool[0:1 + GW, 0:256], lhsT=bm[:], rhs=yy[:],
                                 start=(b == 0), stop=(b == NBC - 1))
                if debug and l == 0 and b == 0:
                    dy_ = blkp.tile([128, 128], F32, tag="dy_")
                    nc.vector.tensor_copy(dy_[:], y)
                    nc.sync.dma_start(out=dbg_y[:], in_=dy_[:])
                if l == 0:
                    ybf = blkp.tile([128, D], BF16, tag="ybf")
                    nc.vector.tensor_copy(ybf[:], y)
                    if b < 25:
                        yo = y_ownA[b * 128:(b + 1) * 128, :]
                    else:
                        yo = y_ownB[(b - 25) * 128:(b - 24) * 128, :]
                    nc.sync.dma_start(out=yo, in_=ybf[:])
                    nc.sync.dma_start(out=y1_res[b * 128:(b + 1) * 128, :], in_=y)
            return stats_ps

        # ---- layer 1
        assert 1 + GW <= 128
        statpool = ps_misc.tile([128, 256], F32, tag="statpool")
        stats1 = layer(0, xA, xB, xbf_in)
        nc.gpsimd.collective_compute(
            "AllGather", ALU.bypass, replica_groups=rg,
            ins=[y_ownA[:, :]], outs=[yA[:, :]])

        # ---- BN1 stats ride the y_ownB AllGather as hi/lo bf16 pairs
        # (the mesh relay only preserves bf16 VALUES, not raw f32 bytes)
        st_sb = work.tile([1, 256], F32, tag="stsb")
        nc.vector.tensor_copy(st_sb[:], stats1[:])
        sthl = work.tile([1, 512], BF16, tag="sthl")
        nc.vector.tensor_copy(sthl[:, 0:256], st_sb[:])
        sthf = work.tile([1, 256], F32, tag="sthf")
        nc.vector.tensor_copy(sthf[:], sthl[:, 0:256])
        stlf = work.tile([1, 256], F32, tag="stlf")
        nc.vector.tensor_tensor(out=stlf[:], in0=st_sb[:], in1=sthf[:],
                                op=ALU.subtract)
        nc.vector.tensor_copy(sthl[:, 256:512], stlf[:])
        nc.sync.dma_start(out=stats_scr[:], in_=sthl[:])
        nc.sync.dma_start(
            out=y_ownB[NPC - HA:HBS, :],
            in_=stats_scr[:].rearrange("o (p c) -> (o p) c", c=128))
        nc.gpsimd.collective_compute(
            "AllGather", ALU.bypass, replica_groups=rg,
            ins=[y_ownB[:, :]], outs=[yB[:, :]])
        # extract + sum the 8 cores' stats rows (all on partition 0 so the
        # 8-way sum can run as exact fp32 DVE adds along the free dim)
        st8 = work.tile([1, 4096], BF16, tag="st8")
        nc.sync.dma_start(
            out=st8[:].rearrange("o (c x) -> o c x", c=8),
            in_=yB[:, :].rearrange("(c r) d -> c r d", r=HBS)
            [:, HBS - 4:HBS, :].rearrange("c r d -> c (r d)"))
        arv = work.tile([1, 256], F32, tag="arv")
        nc.vector.tensor_copy(arv[:], st8[0:1, 0:256])
        for k_ in range(1, 2 * NCORES):
            nc.vector.tensor_tensor(out=arv[:], in0=arv[:],
                                    in1=st8[0:1, k_ * 256:(k_ + 1) * 256],
                                    op=ALU.add)
        if debug:
            nc.sync.dma_start(out=dbg_arv[:], in_=arv[:])
            nc.sync.dma_start(out=dbg_stsb[:], in_=st_sb[:])
            nc.sync.dma_start(out=dbg_st8[:], in_=st8[:])
        mean = work.tile([1, 128], F32, tag="mean")
        nc.vector.tensor_scalar_mul(mean[:], arv[0:1, 0:128], 1.0 / N)
        msq = work.tile([1, 128], F32, tag="msq")
        nc.vector.tensor_scalar_mul(msq[:], arv[0:1, 128:256], 1.0 / N)
        m2 = work.tile([1, 128], F32, tag="m2")
        nc.vector.tensor_tensor(out=m2[:], in0=mean[:], in1=mean[:], op=ALU.mult)
        var = work.tile([1, 128], F32, tag="var")
        nc.vector.tensor_tensor(out=var[:], in0=msq[:], in1=m2[:], op=ALU.subtract)
        lnv = work.tile([1, 128], F32, tag="lnv")
        nc.scalar.activation(lnv[:], var[:], AF.Ln, bias=eps_r[:])
        sraw = work.tile([1, 128], F32, tag="sraw")
        nc.scalar.activation(sraw[:], lnv[:], AF.Exp, scale=neghalf_r[:])
        s1r = work.tile([1, 128], F32, tag="s1r")
        nc.vector.tensor_tensor(out=s1r[:], in0=sraw[:], in1=bng_t[:], op=ALU.mult)
        ms1 = work.tile([1, 128], F32, tag="ms1")
        nc.vector.tensor_tensor(out=ms1[:], in0=mean[:], in1=s1r[:], op=ALU.mult)
        b1r = work.tile([1, 128], F32, tag="b1r")
        nc.vector.tensor_tensor(out=b1r[:], in0=bnb_t[:], in1=ms1[:], op=ALU.subtract)
        # row -> column via DRAM bounce
        nc.sync.dma_start(out=rowscr[0:1, :], in_=s1r[:])
        nc.sync.dma_start(out=rowscr[1:2, :], in_=b1r[:])
        s1c = work.tile([128, 1], F32, tag="s1c")
        nc.sync.dma_start(out=s1c[:], in_=rowscr[0:1, :].rearrange("o (p q) -> (o p) q", q=1))
        b1c = work.tile([128, 1], F32, tag="b1c")
        nc.sync.dma_start(out=b1c[:], in_=rowscr[1:2, :].rearrange("o (p q) -> (o p) q", q=1))
        b1cb = work.tile([128, 1], BF16, tag="b1cb")
        nc.vector.tensor_copy(b1cb[:], b1c[:])
        # S1/B1 broadcast matrices via K=1 outer product
        s1ps = ps_agg.tile([128, 128], F32, tag="agg")
        nc.tensor.matmul(s1ps[:], lhsT=ones_r[:], rhs=s1r[:], start=True, stop=True)
        nc.vector.tensor_copy(S1_t[:], s1ps[:])
        b1ps = ps_agg.tile([128, 128], F32, tag="agg")
        nc.tensor.matmul(b1ps[:], lhsT=ones_r[:], rhs=b1r[:], start=True, stop=True)
        nc.vector.tensor_copy(B1_t[:], b1ps[:])
        # fold BN1 into layer-2 weights: bias row first (raw W), then scale
        wd2, ws2, we2 = W[1]
        fold = statpool[0:1, 0:256]
        nc.tensor.matmul(fold, lhsT=b1cb[:], rhs=wd2[:], start=True, stop=False)
        nc.tensor.matmul(fold, lhsT=b1cb[:], rhs=ws2[:], start=False, stop=True)
        nc.vector.tensor_tensor(out=we2[64:65, :], in0=we2[64:65, :], in1=fold,
                                op=ALU.add)
        nc.vector.tensor_scalar_mul(wd2[:], wd2[:], s1c[:])
        nc.vector.tensor_scalar_mul(ws2[:], ws2[:], s1c[:])
        # ---- layer 2
        stats2 = layer(1, yA, yB, y_ownA, y_ownB)

        # ---- epilogue
        st2 = work.tile([1, 256], F32, tag="st2")
        nc.vector.tensor_copy(st2[:], stats2)
        nc.sync.dma_start(out=stats2_out[:], in_=st2[:])
        pl = work.tile([1 + GW, 128], F32, tag="pl")
        nc.vector.tensor_copy(pl[:], statpool[0:1 + GW, 0:128])
        nc.sync.dma_start(out=pool_out[:], in_=pl[1:1 + GW, :])

    nc.compile()
    return nc


# ---------------------------------------------------------------------------
# entry points
# ---------------------------------------------------------------------------

def _run(inputs, G, trace=False, debug=False):
    x = np.asarray(inputs["x"], np.float32)
    prep = _prep(x, inputs["edge_index"], inputs["edge_attr"], inputs["batch"], G)
    nc = _build(prep, debug=debug)

    wcat = [_wcat(inputs["Wf0"], inputs["bf0"], inputs["Ws0"], inputs["bs0"]),
            _wcat(inputs["Wf1"], inputs["bf1"], inputs["Ws1"], inputs["bs1"])]
    in_maps = []
    for c in range(NCORES):
        im = {
            "x_shard": prep["x_shard"][c],
            "x_shard_bf": prep["x_shard_bf"][c],
            "xAt": prep["xAt"],
            "xBt": prep["xBt"],
            "idx_src": prep["idx_src_w"][c],
            "ea_flat": prep["ea_flat"][c],
            "u_flat": prep["u_flat"][c],
            "ut_flat": prep["ut_flat"][c],
            "mask": prep["mask"][c],
            "Bmat": prep["Bmat"][c],
            "bn_g": np.asarray(inputs["g0"], np.float32).reshape(1, D),
            "bn_b": np.asarray(inputs["be0"], np.float32).reshape(1, D),
        }
        for l in range(2):
            im[f"wd{l}"], im[f"ws{l}"], im[f"we{l}"] = wcat[l]
        in_maps.append(im)

    res = run_bass_kernel_spmd(nc, in_maps, list(range(NCORES)), trace=trace)

    # host combine
    N = prep["N"]
    stats = sum(r["stats2"][0] for r in res.results)
    mean2 = stats[:D] / N
    var2 = stats[D:] / N - mean2 ** 2
    s2 = np.asarray(inputs["g1"], np.float32) / np.sqrt(var2 + EPS)
    b2 = np.asarray(inputs["be1"], np.float32) - mean2 * s2
    GW, glo, cnts = prep["GW"], prep["glo"], prep["cnts"]
    pool = np.zeros((G, D), np.float32)
    for c in range(NCORES):
        g0 = int(glo[c])
        hi = min(G, g0 + GW)
        pool[g0:hi] += res.results[c]["pool_out"][:hi - g0]
    out = np.zeros((G, D), np.float32)
    nz = cnts > 0
    out[nz] = s2[None, :] * pool[nz] / cnts[nz, None] + b2[None, :]
    return out, res


def kernel(**inputs):
    out, _ = _run(inputs, G=256)
    return out
